# revision 20
# baseline (speedup 1.0000x reference)
"""Trainium2 Bass kernel for nn_AttnModule_27152783245631.

Shards batch (16) over 8 NeuronCores (2 per core). Mixed-precision fp8/bf16:
the attention path (pq/pk/pv, E=exp(S/t), denominator, x_update) runs in
fp8-e4m3 with DoubleRow matmuls (2 K-tiles per instruction), and the 3x3 conv
runs with its x_update input half in fp8-DoubleRow and its AdaIN input half in
bf16 (fp8 there would exceed the error budget). All fp8 scaling is folded into
host-side weight prep and the existing affine drains:
  weights Wq/Wk/Wv stored *256 (fp8), pq/pk/pv stored *8 (drain *1/32),
  E stored unscaled fp8 (exp scale T_INV/64), ones=0.5 so rec=2/denom and the
  x_update pad tile comes out *16, conv weights: xu half *16 fp8 / ad half
  *256 bf16 so PSUM is *256, conv drain divides by 256 via the LeakyReLU
  affine constants.
Fast path assumes alpha==1, beta==1 (the provided fills); otherwise a numpy
fallback computes the exact general result on host.
"""

import numpy as np
import ml_dtypes

import concourse.bass as bass
import concourse.tile as tile
from concourse import bacc, mybir
from concourse.bass_utils import run_bass_kernel_spmd

B, C, H, W = 16, 512, 32, 32
N = H * W                  # 1024
CQ = 64
NCORES = 8
BPC = B // NCORES          # batches per core
T_INV = 1.0 / float(CQ) ** 0.5
EPS = 1e-5
NEG = 0.2
PADW = 34                  # padded spatial width
PADN = PADW * PADW         # 1156
BF16 = mybir.dt.bfloat16
F32 = mybir.dt.float32
F8 = mybir.dt.float8e4
DR = mybir.MatmulPerfMode.DoubleRow
NPBF16 = ml_dtypes.bfloat16
NPF8 = ml_dtypes.float8_e4m3
LRELU = False               # fuse conv drain into one scalar Lrelu op

_prog_cache = {}


def _build(reps=1, use_bv=False):
    nc = bacc.Bacc("TRN2", target_bir_lowering=False, debug=False)

    x8 = nc.dram_tensor("x8", [BPC, 128, 4, N], F8, kind="ExternalInput")
    y8 = nc.dram_tensor("y8", [BPC, 128, 4, N], F8, kind="ExternalInput")
    xb = nc.dram_tensor("xb", [BPC, 128, 4, N], BF16, kind="ExternalInput")
    i16 = nc.dram_tensor("i16", [BPC, 128, 4, N], BF16, kind="ExternalInput")
    wq = nc.dram_tensor("wq", [128, 4, 128], F8, kind="ExternalInput")
    wk = nc.dram_tensor("wk", [128, 4, 128], F8, kind="ExternalInput")
    wv = nc.dram_tensor("wv", [128, 4, 512], F8, kind="ExternalInput")
    wc8 = nc.dram_tensor("wc8", [128, 36, 512], F8, kind="ExternalInput")
    wcb = nc.dram_tensor("wcb", [128, 36, 512], BF16, kind="ExternalInput")
    bqk = nc.dram_tensor("bqk", [128, 2], F32, kind="ExternalInput")
    bvv = nc.dram_tensor("bvv", [512], F32, kind="ExternalInput")
    bcc = nc.dram_tensor("bcc", [128, 4], F32, kind="ExternalInput")
    o32 = nc.dram_tensor("o32", [BPC, 4, 128, N], F32, kind="ExternalOutput")

    with tile.TileContext(nc) as tc:
        with tc.tile_pool(name="consts", bufs=1) as consts, \
             tc.tile_pool(name="io", bufs=2) as io, \
             tc.tile_pool(name="work", bufs=2) as work, \
             tc.tile_pool(name="pads", bufs=2) as pads, \
             tc.tile_pool(name="small", bufs=2) as small, \
             tc.tile_pool(name="ostage", bufs=3) as ostage, \
             tc.tile_pool(name="ps", bufs=3, space="PSUM") as ps, \
             tc.tile_pool(name="psd", bufs=1, space="PSUM") as psd, \
             tc.tile_pool(name="psc", bufs=3, space="PSUM") as psc:

            # ---- constants (small first; big conv weights on gpsimd queue) ----
            wq_sb = consts.tile([128, 4, 128], F8)
            nc.sync.dma_start(wq_sb[:], wq[:])
            wk_sb = consts.tile([128, 4, 128], F8)
            nc.sync.dma_start(wk_sb[:], wk[:])
            bqk_sb = consts.tile([128, 2], F32)
            nc.sync.dma_start(bqk_sb[:], bqk[:])
            bcc_sb = consts.tile([128, 4], F32)
            nc.sync.dma_start(bcc_sb[:], bcc[:])
            if not LRELU:
                bc04_sb = consts.tile([128, 4], F32)
                nc.vector.tensor_scalar_mul(bc04_sb[:], bcc_sb[:], 0.4)
                bc06_sb = consts.tile([128, 4], F32)
                nc.vector.tensor_scalar_mul(bc06_sb[:], bcc_sb[:], 0.6)
            if use_bv:
                bv_sb = consts.tile([128, 512], F32)
                nc.sync.dma_start(bv_sb[:],
                                  bvv[None, :].to_broadcast((128, 512)))
            ones_sb = consts.tile([128, 2, 128], F8)
            nc.vector.memset(ones_sb[:], 0.5)
            eps_sb = consts.tile([128, 1], F32)
            nc.vector.memset(eps_sb[:], EPS)
            wv_sb = consts.tile([128, 4, 512], F8)
            nc.gpsimd.dma_start(wv_sb[:], wv[:])
            wc8_sb = consts.tile([128, 36, 512], F8)
            for wkt in range(4):
                nc.gpsimd.dma_start(wc8_sb[:, bass.ts(wkt, 9), :],
                                    wc8[:, bass.ts(wkt, 9), :])
            wcb_sb = consts.tile([128, 36, 512], BF16)
            for wkt in range(4):
                nc.gpsimd.dma_start(wcb_sb[:, bass.ts(wkt, 9), :],
                                    wcb[:, bass.ts(wkt, 9), :])

            bidx = 0
            for _ in range(reps):
                for b in range(BPC):
                    # ---- load activations ----
                    x_sb = io.tile([128, 4, N], F8, tag="x_sb")
                    y_sb = io.tile([128, 4, N], F8, tag="y_sb")
                    xb_sb = io.tile([128, 4, N], BF16, tag="xb_sb")
                    i_sb = io.tile([128, 4, N], BF16, tag="i_sb")
                    for kt in range(4):
                        nc.sync.dma_start(x_sb[:, kt, :], x8[b, :, kt, :])
                    for kt in range(4):
                        nc.sync.dma_start(y_sb[:, kt, :], y8[b, :, kt, :])
                    for kt in range(4):
                        nc.sync.dma_start(xb_sb[:, kt, :], xb[b, :, kt, :])
                    for kt in range(4):
                        nc.sync.dma_start(i_sb[:, kt, :], i16[b, :, kt, :])

                    # ---- pad buffers (borders stay zero; interior is fully
                    #      overwritten each use, so zero borders only once
                    #      per physical buffer) ----
                    pad8 = pads.tile([128, 4, PADN], F8, tag="pad8")
                    padb = pads.tile([128, 4, PADN], BF16, tag="padb")
                    pad8_v = pad8.rearrange("p k (a b) -> p k a b", a=PADW)
                    padb_v = padb.rearrange("p k (a b) -> p k a b", a=PADW)
                    if bidx < 2:
                        for pv_ in (pad8_v, padb_v):
                            nc.gpsimd.memset(pv_[:, :, 0, :], 0.0)
                            nc.gpsimd.memset(pv_[:, :, 33, :], 0.0)
                            nc.gpsimd.memset(pv_[:, :, :, 0], 0.0)
                            nc.gpsimd.memset(pv_[:, :, :, 33], 0.0)
                    bidx += 1

                    # ---- pq = fp8(8(Wq x + bq)) ; pk from y (DoubleRow) ----
                    pq_sb = work.tile([128, N], F8, tag="pq")
                    pk_sb = work.tile([128, N], F8, tag="pk")
                    for (dst, wsb, src, bcol) in ((pq_sb, wq_sb, x_sb, 0),
                                                  (pk_sb, wk_sb, y_sb, 1)):
                        for mc in range(2):
                            pt = ps.tile([128, 512], F32, tag="acc")
                            for j in range(2):
                                nc.tensor.matmul(
                                    pt[:], wsb[:, 2 * j:2 * j + 2, :],
                                    src[:, 2 * j:2 * j + 2, bass.ts(mc, 512)],
                                    start=(j == 0), stop=(j == 1),
                                    perf_mode=DR)
                            nc.vector.tensor_scalar(
                                dst[:, bass.ts(mc, 512)], pt[:],
                                1.0 / 32.0, bqk_sb[:, bcol:bcol + 1],
                                op0=mybir.AluOpType.mult,
                                op1=mybir.AluOpType.add)

                    # ---- pv_T[n, c] = fp8(8(Wv x + bv)^T) (DoubleRow) ----
                    pvT_sb = work.tile([128, 8, 512], F8, tag="pvT")
                    for nt in range(8):
                        pt = ps.tile([128, 512], F32, tag="acc")
                        for j in range(2):
                            nc.tensor.matmul(
                                pt[:],
                                x_sb[:, 2 * j:2 * j + 2, bass.ts(nt, 128)],
                                wv_sb[:, 2 * j:2 * j + 2, :],
                                start=(j == 0), stop=(j == 1),
                                perf_mode=DR)
                        if use_bv:
                            tmp = small.tile([128, 512], F32, tag="tmp")
                            nc.vector.tensor_scalar_mul(
                                tmp[:], pt[:], 1.0 / 32.0)
                            nc.vector.tensor_tensor(
                                pvT_sb[:, nt, :], tmp[:], bv_sb[:],
                                mybir.AluOpType.add)
                        else:
                            nc.vector.tensor_scalar_mul(
                                pvT_sb[:, nt, :], pt[:], 1.0 / 32.0)

                    # ---- E = fp8(exp(S/t)), denom = 0.5*sum_n E (DoubleRow) ----
                    e_sb = work.tile([128, 8, N], F8, tag="e_sb")
                    den_ps = [psd.tile([128, 512], F32, tag="den_ps",
                                       name=f"den_ps_{mc}")
                              for mc in range(2)]
                    for nt in range(8):
                        for mc in range(2):
                            et = ps.tile([128, 512], F32, tag="acc")
                            nc.tensor.matmul(
                                et[:], pk_sb[:, bass.ts(nt, 128)],
                                pq_sb[:, bass.ts(mc, 512)],
                                start=True, stop=True)
                            nc.scalar.activation(
                                e_sb[:, nt, bass.ts(mc, 512)], et[:],
                                mybir.ActivationFunctionType.Exp,
                                bias=0.0, scale=T_INV / 64.0)
                        if nt % 2 == 1:
                            for mh in range(2):
                                nc.tensor.matmul(
                                    den_ps[mh][:], ones_sb[:],
                                    e_sb[:, nt - 1:nt + 1, bass.ts(mh, 512)],
                                    start=(nt == 1), stop=(nt == 7),
                                    perf_mode=DR)
                    rec_sb = work.tile([128, N], F32, tag="rec")
                    for mc in range(2):
                        nc.vector.reciprocal(rec_sb[:, bass.ts(mc, 512)],
                                             den_ps[mc][:])
                    rec_v = rec_sb.rearrange("p (a b) -> p a b", a=32)

                    # ---- x_update*16 -> pad8 rows (DoubleRow) ----
                    for ct in range(4):
                        for mc in range(2):
                            xt = ps.tile([128, 512], F32, tag="acc")
                            for j in range(4):
                                nc.tensor.matmul(
                                    xt[:],
                                    pvT_sb[:, 2 * j:2 * j + 2, bass.ts(ct, 128)],
                                    e_sb[:, 2 * j:2 * j + 2, bass.ts(mc, 512)],
                                    start=(j == 0), stop=(j == 3),
                                    perf_mode=DR)
                            nc.vector.tensor_tensor(
                                pad8_v[:, ct,
                                       1 + mc * 16:1 + mc * 16 + 16, 1:33],
                                xt[:].rearrange("p (a b) -> p a b", a=16),
                                rec_v[:, mc * 16:mc * 16 + 16, :],
                                mybir.AluOpType.mult)

                    # ---- AdaIN -> padb rows (bf16) ----
                    for ct in range(4):
                        stx = small.tile([128, 2, 6], F32, tag="stx")
                        nc.vector.bn_stats(out=stx[:, 0, :], in_=xb_sb[:, ct, 0:512])
                        nc.vector.bn_stats(out=stx[:, 1, :], in_=xb_sb[:, ct, 512:N])
                        mvx = small.tile([128, 2], F32, tag="mvx")
                        nc.vector.bn_aggr(out=mvx[:], in_=stx[:])
                        sti = small.tile([128, 2, 6], F32, tag="sti")
                        nc.vector.bn_stats(out=sti[:, 0, :], in_=i_sb[:, ct, 0:512])
                        nc.vector.bn_stats(out=sti[:, 1, :], in_=i_sb[:, ct, 512:N])
                        mvi = small.tile([128, 2], F32, tag="mvi")
                        nc.vector.bn_aggr(out=mvi[:], in_=sti[:])
                        tstd = small.tile([128, 1], F32, tag="tstd")
                        nc.scalar.activation(
                            tstd[:], mvx[:, 1:2],
                            mybir.ActivationFunctionType.Sqrt,
                            bias=eps_sb[:], scale=float(N) / (N - 1))
                        istd = small.tile([128, 1], F32, tag="istd")
                        nc.scalar.activation(
                            istd[:], mvi[:, 1:2],
                            mybir.ActivationFunctionType.Sqrt,
                            bias=eps_sb[:], scale=float(N) / (N - 1))
                        irstd = small.tile([128, 1], F32, tag="irstd")
                        nc.vector.reciprocal(irstd[:], istd[:])
                        scale = small.tile([128, 1], F32, tag="scale")
                        nc.vector.tensor_tensor(scale[:], tstd[:], irstd[:],
                                                mybir.AluOpType.mult)
                        shift = small.tile([128, 1], F32, tag="shift")
                        nc.vector.tensor_tensor(shift[:], mvi[:, 0:1], scale[:],
                                                mybir.AluOpType.mult)
                        nc.vector.tensor_tensor(shift[:], mvx[:, 0:1], shift[:],
                                                mybir.AluOpType.subtract)
                        nc.vector.tensor_scalar(
                            padb_v[:, ct, 1:33, 1:33],
                            i_sb[:, ct].rearrange("p (a b) -> p a b", a=32),
                            scale[:], shift[:],
                            op0=mybir.AluOpType.mult, op1=mybir.AluOpType.add)

                    # ---- conv 3x3 + LeakyReLU: 36 bf16 + 18 fp8-DR matmuls
                    #      per [128,512] psum tile; PSUM is 256x the result ----
                    for mt in range(4):
                        for hq in range(2):
                            pc = psc.tile([128, 512], F32, tag="conv_ps")
                            for tap in range(9):
                                ky, kx = tap // 3, tap % 3
                                for kt in range(4):
                                    nc.tensor.matmul(
                                        pc[:],
                                        wcb_sb[:, tap * 4 + kt, bass.ts(mt, 128)],
                                        padb_v[:, kt,
                                               hq * 16 + ky:hq * 16 + ky + 16,
                                               kx:kx + 32],
                                        start=(tap == 0 and kt == 0),
                                        stop=False)
                            for tap in range(9):
                                ky, kx = tap // 3, tap % 3
                                for j in range(2):
                                    nc.tensor.matmul(
                                        pc[:],
                                        wc8_sb[:, tap * 4 + 2 * j:
                                               tap * 4 + 2 * j + 2,
                                               bass.ts(mt, 128)],
                                        pad8_v[:, 2 * j:2 * j + 2,
                                               hq * 16 + ky:hq * 16 + ky + 16,
                                               kx:kx + 32],
                                        start=False,
                                        stop=(tap == 8 and j == 1),
                                        perf_mode=DR)
                            if LRELU:
                                ot = ostage.tile([128, 512], F32, tag="ot")
                                nc.scalar.activation(
                                    ot[:], pc[:],
                                    mybir.ActivationFunctionType.Lrelu,
                                    bias=bcc_sb[:, mt:mt + 1],
                                    scale=1.0 / 256.0, alpha=NEG)
                            else:
                                ab = ostage.tile([128, 512], F32, tag="ab")
                                nc.scalar.activation(
                                    ab[:], pc[:],
                                    mybir.ActivationFunctionType.Abs,
                                    bias=bc04_sb[:, mt:mt + 1],
                                    scale=0.4 / 256.0)
                                ot = ostage.tile([128, 512], F32, tag="ot")
                                nc.vector.tensor_scalar(
                                    ot[:], pc[:],
                                    0.6 / 256.0, bc06_sb[:, mt:mt + 1],
                                    op0=mybir.AluOpType.mult,
                                    op1=mybir.AluOpType.add)
                                nc.vector.tensor_tensor(
                                    ot[:], ot[:], ab[:], mybir.AluOpType.add)
                            nc.sync.dma_start(
                                o32[b, mt, :, bass.ts(hq, 512)], ot[:])

    nc.finalize()
    return nc


def _get_prog(reps=1, use_bv=False):
    key = (reps, use_bv)
    if key not in _prog_cache:
        _prog_cache[key] = _build(reps, use_bv)
    return _prog_cache[key]


def _prep_in_maps(inp, x, y, Wq, bq, Wk, bk, Wv, bv, Wc, bc):
    def act_prep(a, npdt):
        # [B,C,H,W] f32 -> [B,128,4,N] (c = ct*128 + p)
        return np.ascontiguousarray(
            a.reshape(B, 4, 128, N).transpose(0, 2, 1, 3)).astype(npdt)

    x8_h = act_prep(x, NPF8)
    y8_h = act_prep(y, NPF8)
    xb_h = act_prep(x, NPBF16)
    i16_h = act_prep(inp, NPBF16)

    def wqk_prep(w):
        wp = np.zeros((128, C), np.float32)
        wp[:CQ] = w * 256.0
        return np.ascontiguousarray(
            wp.T.reshape(4, 128, 128).transpose(1, 0, 2)).astype(NPF8)

    wq_h = wqk_prep(Wq)
    wk_h = wqk_prep(Wk)
    wv_h = np.ascontiguousarray(
        (Wv * 256.0).T.reshape(4, 128, 512).transpose(1, 0, 2)).astype(NPF8)

    def wc_prep(whalf, s, npdt):
        # [512o, 512i, 3, 3] -> [128p, 36(tap*4+kt), 512o]
        return np.ascontiguousarray(
            (whalf * s).transpose(2, 3, 1, 0).reshape(9, 4, 128, 512)
            .transpose(2, 0, 1, 3).reshape(128, 36, 512)).astype(npdt)

    wc8_h = wc_prep(Wc[:, :C], 16.0, NPF8)
    wcb_h = wc_prep(Wc[:, C:], 256.0, NPBF16)
    bqk_h = np.zeros((128, 2), np.float32)
    bqk_h[:CQ, 0] = 8.0 * bq
    bqk_h[:CQ, 1] = 8.0 * bk
    bvv_h = 8.0 * bv.astype(np.float32)
    bcc_h = np.ascontiguousarray(bc.reshape(4, 128).T).astype(np.float32)

    in_maps = []
    for c in range(NCORES):
        s = slice(c * BPC, (c + 1) * BPC)
        in_maps.append({
            "x8": x8_h[s], "y8": y8_h[s], "xb": xb_h[s], "i16": i16_h[s],
            "wq": wq_h, "wk": wk_h, "wv": wv_h, "wc8": wc8_h, "wcb": wcb_h,
            "bqk": bqk_h, "bvv": bvv_h, "bcc": bcc_h,
        })
    return in_maps


def _assemble(results):
    out = np.empty((B, C, H, W), np.float32)
    for c in range(NCORES):
        o = results[c]["o32"]  # [BPC, 4, 128, N]
        out[c * BPC:(c + 1) * BPC] = o.reshape(BPC, C, H, W)
    return out


def _np_reference(inp, x, y, Wq, bq, Wk, bk, Wv, bv, Wc, bc, alpha, beta):
    # Exact general-path fallback on host (numpy, fp32).
    b, c, h, w = x.shape
    n = h * w
    t = float(CQ) ** 0.5

    def conv1x1(a, Wm, bb):
        return (np.einsum("oc,bcn->bon", Wm, a.reshape(b, c, n))
                + bb[None, :, None])

    def softmax(s):
        s = s - s.max(axis=-1, keepdims=True)
        e = np.exp(s)
        return e / e.sum(axis=-1, keepdims=True)

    pq = conv1x1(x, Wq, bq)
    pk = conv1x1(y, Wk, bk)
    pv = conv1x1(x, Wv, bv)
    attn_iden = softmax(np.einsum("bcn,bcm->bnm", pq, pk) / t)
    pq_p = conv1x1(y, Wq, bq)
    pk_p = conv1x1(x, Wk, bk)
    attn_pose = softmax(np.einsum("bcn,bcm->bnm", pq_p, pk_p) / t)
    xu = np.einsum("bcn,bmn->bcm", pv, attn_iden).reshape(b, c, h, w)
    xu = (1.0 - beta) * x + beta * xu

    def mean_std(f):
        v = f.reshape(b, c, n)
        m = v.mean(axis=2)
        s = np.sqrt(v.var(axis=2, ddof=1) + EPS)
        return m[:, :, None], s[:, :, None]

    tm, ts_ = mean_std(x)
    im, is_ = mean_std(inp)
    ad = ((inp.reshape(b, c, n) - im) / is_ * ts_ + tm)
    rev = np.einsum("bcn,bmn->bcm", ad, 1.0 - attn_pose)
    rev = (1.0 - alpha) * rev + alpha * ad
    cat = np.concatenate([xu.reshape(b, c, n), rev], axis=1).reshape(
        b, 2 * c, h, w)
    catp = np.pad(cat, ((0, 0), (0, 0), (1, 1), (1, 1)))
    out = np.zeros((b, c, h, w), np.float32)
    for ky in range(3):
        for kx in range(3):
            out += np.einsum("oi,bihw->bohw", Wc[:, :, ky, kx],
                             catp[:, :, ky:ky + h, kx:kx + w])
    out += bc[None, :, None, None]
    return np.where(out >= 0, out, NEG * out).astype(np.float32)


def _run(in_maps, reps=1, use_bv=False):
    nc = _get_prog(reps, use_bv)
    return run_bass_kernel_spmd(nc, in_maps, list(range(NCORES)))


def kernel(inp, x, y, Wq, bq, Wk, bk, Wv, bv, Wc, bc, alpha, beta):
    args = [np.asarray(a, np.float32) for a in
            (inp, x, y, Wq, bq, Wk, bk, Wv, bv, Wc, bc)]
    alpha = np.asarray(alpha, np.float32)
    beta = np.asarray(beta, np.float32)
    if float(alpha.reshape(-1)[0]) != 1.0 or float(beta.reshape(-1)[0]) != 1.0:
        return _np_reference(*args, alpha.reshape(-1)[0], beta.reshape(-1)[0])
    use_bv = bool(np.any(args[8]))
    in_maps = _prep_in_maps(*args)
    res = _run(in_maps, use_bv=use_bv)
    return _assemble(res.results)
